# revision 1
# baseline (speedup 1.0000x reference)
"""Trainium2 Bass kernel for nn_CombinedModel (NMS detection + ROI classifier).

Sharding: pooled-pixel-row sharding. Core c computes conv output rows
y in [8c, 8c+8) (= pooled rows py in [4c,4c+4)) of ALL 300 ROIs, which is
exactly the k-slice S_c = {(oc, py, px): py in [4c,4c+4)} of the 16384-wide
W1 contraction. Each core runs the 8-head GEMM against its 2048-row W1
slice, a ReduceScatter sums the partial [8,128,304] and hands head c to
core c, which applies bias/relu + its head matmul + keep mask.
NMS / top-k / ROI selection is tiny and done host-side during input prep.
"""
import numpy as np

N_CORES = 8
R = 304            # 300 rois padded to 8*38
IMG = 640
INP = 64
CONF = 0.25
IOU = 0.45
K = 300
PROV, ALPHA, AD = 38, 25, 35
OUTW = 40          # padded per-core head width

_CACHE = {}


def _build_bass():
    import concourse.bacc as bacc
    import concourse.mybir as mybir
    import concourse.tile as tile

    nc = bacc.Bacc("TRN2", target_bir_lowering=False, debug=False,
                   num_devices=N_CORES)
    f32 = mybir.dt.float32
    cols = nc.dram_tensor("cols", [108, 38912], f32, kind="ExternalInput").ap()
    wstk = nc.dram_tensor("wstk", [108, 64], f32, kind="ExternalInput").ap()
    bc64 = nc.dram_tensor("bc64", [64, 1], f32, kind="ExternalInput").ap()
    w1s = nc.dram_tensor("w1s", [8, 16, 128, 128], f32, kind="ExternalInput").ap()
    b1c = nc.dram_tensor("b1c", [128, 1], f32, kind="ExternalInput").ap()
    w2 = nc.dram_tensor("w2", [128, OUTW], f32, kind="ExternalInput").ap()
    b2 = nc.dram_tensor("b2", [OUTW, 1], f32, kind="ExternalInput").ap()
    keepm = nc.dram_tensor("keepm", [OUTW, R], f32, kind="ExternalInput").ap()
    out = nc.dram_tensor("out", [OUTW, R], f32, kind="ExternalOutput").ap()

    NM = 76  # conv matmuls of 512 cols each

    with tile.TileContext(nc) as tc:
        with (
            tc.tile_pool(name="const", bufs=1) as cpool,
            tc.tile_pool(name="colsp", bufs=3) as colsp,
            tc.tile_pool(name="psum", bufs=1, space="PSUM") as psum,
            tc.tile_pool(name="work", bufs=2) as work,
            tc.tile_pool(name="dram", bufs=1, space="DRAM") as dpool,
        ):
            wstk_sb = cpool.tile([108, 64], f32)
            nc.sync.dma_start(wstk_sb[:], wstk[:])
            bc64_sb = cpool.tile([64, 1], f32)
            nc.sync.dma_start(bc64_sb[:], bc64[:])
            b1c_sb = cpool.tile([128, 1], f32)
            nc.sync.dma_start(b1c_sb[:], b1c[:])
            w2_sb = cpool.tile([128, OUTW], f32)
            nc.sync.dma_start(w2_sb[:], w2[:])
            b2_sb = cpool.tile([OUTW, 1], f32)
            nc.sync.dma_start(b2_sb[:], b2[:])
            keep_sb = cpool.tile([OUTW, R], f32)
            nc.sync.dma_start(keep_sb[:], keepm[:])

            pooled2 = cpool.tile([128, 16, R], f32)

            # conv + pool: 4 col chunks of 19 matmuls each
            CH = 19
            for ch in range(4):
                ctile = colsp.tile([108, CH * 512], f32, tag="cols", bufs=2)
                nc.sync.dma_start(ctile[:], cols[:, ch * CH * 512:(ch + 1) * CH * 512])
                for j in range(CH):
                    m = ch * CH + j
                    ps = psum.tile([64, 4, 2, 64], f32, tag="cv", bufs=4)
                    nc.tensor.matmul(ps.rearrange("p a b c -> p (a b c)"),
                                     wstk_sb[:], ctile[:, j * 512:(j + 1) * 512],
                                     start=True, stop=True)
                    craw = work.tile([64, 4, 2, 64], f32, tag="craw")
                    nc.scalar.activation(
                        craw.rearrange("p a b c -> p (a b c)"),
                        ps.rearrange("p a b c -> p (a b c)"),
                        mybir.ActivationFunctionType.Relu,
                        bias=bc64_sb[:])
                    t0 = work.tile([64, 4, 32], f32, tag="t0")
                    t1 = work.tile([64, 4, 32], f32, tag="t1")
                    nc.vector.tensor_tensor(out=t0[:], in0=craw[:, :, 0, 0::2],
                                            in1=craw[:, :, 0, 1::2],
                                            op=mybir.AluOpType.max)
                    nc.vector.tensor_tensor(out=t1[:], in0=craw[:, :, 1, 0::2],
                                            in1=craw[:, :, 1, 1::2],
                                            op=mybir.AluOpType.max)
                    nc.vector.tensor_tensor(
                        out=pooled2[0:64, :, 4 * m:4 * m + 4].rearrange(
                            "p x r -> p r x"),
                        in0=t0[:, :, 0::2], in1=t1[:, :, 0::2],
                        op=mybir.AluOpType.max)
                    nc.vector.tensor_tensor(
                        out=pooled2[64:128, :, 4 * m:4 * m + 4].rearrange(
                            "p x r -> p r x"),
                        in0=t0[:, :, 1::2], in1=t1[:, :, 1::2],
                        op=mybir.AluOpType.max)

            # 8-head GEMM over this core's 2048-row W1 slice
            import os
            STAGE = int(os.environ.get("KSTAGE", "3"))

            if STAGE == 0:
                om0 = work.tile([OUTW, R], f32, tag="om")
                nc.vector.tensor_copy(om0[:], pooled2[:OUTW, 0, :])
                nc.sync.dma_start(out[:], om0[:])
            if STAGE >= 1:
                parts = cpool.tile([128, 8, R], f32)
                for h in range(8):
                    w1h = colsp.tile([128, 16, 128], f32, tag="w1h", bufs=2)
                    nc.sync.dma_start(w1h[:], w1s[h].rearrange("q k d -> k q d"))
                    ph = psum.tile([128, R], f32, tag="gemm", bufs=2)
                    for q in range(16):
                        nc.tensor.matmul(ph[:], w1h[:, q, :], pooled2[:, q, :],
                                         start=(q == 0), stop=(q == 15))
                    nc.vector.tensor_copy(parts[:, h, :], ph[:])
            if STAGE == 1:
                om1 = work.tile([OUTW, R], f32, tag="om")
                nc.vector.tensor_copy(om1[:], parts[:OUTW, 0, :])
                nc.sync.dma_start(out[:], om1[:])
            if STAGE >= 2:
                cc_in = dpool.tile([8, 128, R], f32)
                cc_out = dpool.tile([128, R], f32)
                nc.sync.dma_start(cc_in.rearrange("h p r -> p h r"), parts[:])
                nc.gpsimd.collective_compute(
                    "ReduceScatter", mybir.AluOpType.add,
                    ins=[cc_in[:]], outs=[cc_out[:]],
                    replica_groups=[list(range(N_CORES))],
                )
                hsb = work.tile([128, R], f32, tag="hsb")
                nc.sync.dma_start(hsb[:], cc_out[:])
                hrelu = work.tile([128, R], f32, tag="hrelu")
                nc.scalar.activation(hrelu[:], hsb[:],
                                     mybir.ActivationFunctionType.Relu,
                                     bias=b1c_sb[:])
                po = psum.tile([OUTW, R], f32, tag="head")
                nc.tensor.matmul(po[:], w2_sb[:], hrelu[:], start=True, stop=True)
                ob = work.tile([OUTW, R], f32, tag="ob")
                nc.vector.tensor_scalar(ob[:], po[:], b2_sb[:], None,
                                        op0=mybir.AluOpType.add)
                om = work.tile([OUTW, R], f32, tag="om")
                nc.vector.tensor_tensor(out=om[:], in0=ob[:], in1=keep_sb[:],
                                        op=mybir.AluOpType.mult)
                nc.sync.dma_start(out[:], om[:])
    nc.compile()
    return nc


def _host_prep(preds, image, W_conv, b_conv, W1, b1, W2p, b2p, W2a, b2a, W2d, b2d):
    p = preds[0].astype(np.float32)
    score = p[:, 4] * p[:, 5]
    masked = np.where(score > CONF, score, -np.inf)
    idx = np.argsort(-masked, kind="stable")[:K]
    top_s = masked[idx]
    xy, wh = p[:, 0:2], p[:, 2:4]
    boxes = np.concatenate([xy - wh / 2, xy + wh / 2], axis=-1)
    b = boxes[idx]
    valid = top_s > CONF
    x1, y1, x2, y2 = b[:, 0], b[:, 1], b[:, 2], b[:, 3]
    area = (x2 - x1) * (y2 - y1)
    iw = np.clip(np.minimum(x2[:, None], x2[None, :]) - np.maximum(x1[:, None], x1[None, :]), 0, None)
    ih = np.clip(np.minimum(y2[:, None], y2[None, :]) - np.maximum(y1[:, None], y1[None, :]), 0, None)
    iou = iw * ih / (area[:, None] + area[None, :] - iw * ih + 1e-7)
    keep = valid.copy()
    for i in range(K):
        sup = np.any((iou[i, :i] > IOU) & keep[:i])
        keep[i] = keep[i] & ~sup

    xi = np.clip(np.round(b[:, 0]).astype(np.int32), 0, IMG - INP)
    yi = np.clip(np.round(b[:, 1]).astype(np.int32), 0, IMG - INP)
    img0 = image[0]
    pad = np.zeros((R, 3, 66, 66), np.float32)
    for r in range(K):
        pad[r, :, 1:65, 1:65] = img0[:, yi[r]:yi[r] + 64, xi[r]:xi[r] + 64]

    from numpy.lib.stride_tricks import sliding_window_view
    # patches[roi, c, yy, x, ky, kx]
    patches = sliding_window_view(pad, (3, 3), axis=(2, 3))
    P2 = np.ascontiguousarray(patches.transpose(2, 1, 4, 5, 0, 3))  # [yy,c,ky,kx,roi,x]
    cols_all = np.ascontiguousarray(
        P2.reshape(8, 4, 2, 27, R, 64).transpose(0, 1, 3, 4, 2, 5)
    ).reshape(8, 108, 38912)

    wstk = np.zeros((108, 64), np.float32)
    wc = W_conv.reshape(16, 27).T  # [27, 16]
    for ph in range(4):
        wstk[ph * 27:(ph + 1) * 27, ph * 16:(ph + 1) * 16] = wc
    bc64 = np.tile(b_conv.astype(np.float32), 4).reshape(64, 1)

    # w1s[core][h, px, py*16+oc, d]
    W1r = W1.reshape(8, 16, 32, 32, 128)  # [h, oc, py, px, d]
    w1s_all = np.empty((8, 8, 16, 128, 128), np.float32)
    for core in range(8):
        blk = W1r[:, :, 4 * core:4 * core + 4, :, :]       # [h, oc, py4, px, d]
        t = np.ascontiguousarray(blk.transpose(0, 3, 2, 1, 4))  # [h, px, py, oc, d]
        w1s_all[core] = t.reshape(8, 16, 2, 64, 128).reshape(8, 16, 128, 128)

    w2_all = np.zeros((8, 128, OUTW), np.float32)
    b2_all = np.zeros((8, OUTW, 1), np.float32)
    w2_all[0, :, :PROV] = W2p; b2_all[0, :PROV, 0] = b2p
    w2_all[1, :, :ALPHA] = W2a; b2_all[1, :ALPHA, 0] = b2a
    for j in range(6):
        w2_all[2 + j, :, :AD] = W2d[j]; b2_all[2 + j, :AD, 0] = b2d[j]

    keepf = np.zeros((R,), np.float32)
    keepf[:K] = keep.astype(np.float32)
    keepm = np.broadcast_to(keepf, (OUTW, R)).copy()

    in_maps = []
    for core in range(8):
        in_maps.append({
            "cols": cols_all[core],
            "wstk": wstk,
            "bc64": bc64,
            "w1s": w1s_all[core],
            "b1c": b1[core].reshape(128, 1).astype(np.float32),
            "w2": w2_all[core],
            "b2": b2_all[core],
            "keepm": keepm,
        })
    return in_maps


def kernel(**inputs):
    from concourse import bass_utils
    if "nc" not in _CACHE:
        _CACHE["nc"] = _build_bass()
    nc = _CACHE["nc"]
    in_maps = _host_prep(**{k: np.asarray(v) for k, v in inputs.items()})
    res = bass_utils.run_bass_kernel_spmd(nc, in_maps, core_ids=list(range(N_CORES)))
    _CACHE["last_res"] = res
    outs = [res.results[c]["out"].T for c in range(N_CORES)]  # [304, 40] each
    logits = np.concatenate(
        [outs[0][:K, :PROV], outs[1][:K, :ALPHA]]
        + [outs[2 + j][:K, :AD] for j in range(6)], axis=1)
    return logits.astype(np.float32)



# revision 8
# speedup vs baseline: 1.7934x; 1.7934x over previous
"""Trainium2 Bass kernel for nn_CombinedModel (NMS detection + ROI classifier).

Sharding: pooled-pixel-row sharding. Core c computes conv output rows
y in [8c, 8c+8) (= pooled rows py in [4c,4c+4)) of ALL 300 ROIs, which is
exactly the k-slice S_c = {(oc, py, px): py in [4c,4c+4)} of the 16384-wide
W1 contraction. Each core runs the 8-head GEMM against its 2048-row W1
slice, a ReduceScatter sums the partial [8,128,304] and hands head c to
core c, which applies bias/relu + its head matmul + keep mask.
NMS / top-k / ROI selection is tiny and done host-side during input prep.

v2: bf16 matmuls (4x PE), conv bias folded into matmul via extra K rows,
pool-before-relu with relu fused into the x-pool scalar_tensor_tensor,
y-pool on GpSimd, contiguous pooling layout, bf16 ReduceScatter payload.
"""
import numpy as np

N_CORES = 8
R = 304            # 300 rois padded to 8*38
IMG = 640
INP = 64
CONF = 0.25
IOU = 0.45
K = 300
PROV, ALPHA, AD = 38, 25, 35
OUTW = 40          # padded per-core head width
NM = 76            # conv matmuls of 512 cols each (4 rois apiece)

_CACHE = {}


def _build_bass():
    import concourse.bacc as bacc
    import concourse.mybir as mybir
    import concourse.tile as tile

    nc = bacc.Bacc("TRN2", target_bir_lowering=False, debug=False,
                   num_devices=N_CORES)
    f32 = mybir.dt.float32
    bf16 = mybir.dt.bfloat16
    cols = nc.dram_tensor("cols", [112, 38912], bf16, kind="ExternalInput").ap()
    wstk = nc.dram_tensor("wstk", [112, 64], bf16, kind="ExternalInput").ap()
    w1s = nc.dram_tensor("w1s", [8, 128, 16, 128], bf16, kind="ExternalInput").ap()
    b1c = nc.dram_tensor("b1c", [128, 1], f32, kind="ExternalInput").ap()
    w2 = nc.dram_tensor("w2", [128, OUTW], bf16, kind="ExternalInput").ap()
    b2 = nc.dram_tensor("b2", [OUTW, 1], f32, kind="ExternalInput").ap()
    keepm = nc.dram_tensor("keepm", [OUTW, R], f32, kind="ExternalInput").ap()
    out = nc.dram_tensor("out", [OUTW, R], f32, kind="ExternalOutput").ap()

    with tile.TileContext(nc) as tc:
        with (
            tc.tile_pool(name="const", bufs=1) as cpool,
            tc.tile_pool(name="psum", bufs=1, space="PSUM") as psum,
            tc.tile_pool(name="work", bufs=2) as work,
            tc.tile_pool(name="dram", bufs=1, space="DRAM") as dpool,
        ):
            # small constants first (cheap, unblock tail setup)
            wstk_sb = cpool.tile([112, 64], bf16)
            nc.sync.dma_start(wstk_sb[:], wstk[:])
            b1c_sb = cpool.tile([128, 1], f32)
            nc.sync.dma_start(b1c_sb[:], b1c[:])
            w2_sb = cpool.tile([128, OUTW], bf16)
            nc.sync.dma_start(w2_sb[:], w2[:])
            b2_sb = cpool.tile([OUTW, 1], f32)
            nc.sync.dma_start(b2_sb[:], b2[:])
            keep_sb = cpool.tile([OUTW, R], f32)
            nc.sync.dma_start(keep_sb[:], keepm[:])

            # whole im2col matrix persistent in SBUF, DMA'd in col-chunks
            cols_sb = cpool.tile([112, 38912], bf16)
            NCH = 8
            CW = 38912 // NCH
            for ch in range(NCH):
                nc.sync.dma_start(cols_sb[:, ch * CW:(ch + 1) * CW],
                                  cols[:, ch * CW:(ch + 1) * CW])
            # W1 k-slice, one DMA per head so GEMM heads start as they land
            w1_sb = cpool.tile([128, 8, 16, 128], bf16)
            for h in range(8):
                nc.sync.dma_start(w1_sb[:, h], w1s[h])

            pooled2 = cpool.tile([128, 16, R], bf16)

            # conv: matmul m covers 4 rois; PSUM free layout
            # (sub2, b2, l2, px16, r4); bias rides rows 108..111 of wstk/cols
            for m in range(NM):
                ps = psum.tile([64, 512], f32, tag="cv", bufs=4)
                nc.tensor.matmul(ps[:], wstk_sb[:],
                                 cols_sb[:, m * 512:(m + 1) * 512],
                                 start=True, stop=True)
                m0 = work.tile([64, 256], f32, tag="m0", bufs=3)
                nc.scalar.copy(m0[:], ps[:, 0:256])
                m01 = work.tile([64, 256], bf16, tag="m01", bufs=3)
                # y-pool + relu fused: max(ps1, 0, ps0) = relu(max(ps0, ps1))
                nc.vector.scalar_tensor_tensor(
                    out=m01[:], in0=ps[:, 256:512], scalar=0.0, in1=m0[:],
                    op0=mybir.AluOpType.max, op1=mybir.AluOpType.max)
                m01v = m01.rearrange("p (b l x r) -> p b l x r", b=2, l=2, x=16)
                nc.vector.tensor_tensor(
                    out=pooled2[0:64, :, 4 * m:4 * m + 4],
                    in0=m01v[:, 0, 0], in1=m01v[:, 0, 1],
                    op=mybir.AluOpType.max)
                nc.vector.tensor_tensor(
                    out=pooled2[64:128, :, 4 * m:4 * m + 4],
                    in0=m01v[:, 1, 0], in1=m01v[:, 1, 1],
                    op=mybir.AluOpType.max)

            # 8-head GEMM over this core's 2048-row W1 slice
            cc_in = dpool.tile([8, 128, R], bf16)
            cc_out = dpool.tile([128, R], bf16)
            for h in range(8):
                ph = psum.tile([128, R], f32, tag="gemm", bufs=2)
                for q in range(16):
                    nc.tensor.matmul(ph[:], w1_sb[:, h, q, :], pooled2[:, q, :],
                                     start=(q == 0), stop=(q == 15))
                pb = work.tile([128, R], bf16, tag="pb", bufs=2)
                nc.scalar.copy(pb[:], ph[:])
                nc.sync.dma_start(cc_in[h], pb[:])

            nc.gpsimd.collective_compute(
                "ReduceScatter", mybir.AluOpType.add,
                ins=[cc_in[:]], outs=[cc_out[:]],
                replica_groups=[list(range(N_CORES))],
            )
            hsb = work.tile([128, R], bf16, tag="hsb")
            nc.sync.dma_start(hsb[:], cc_out[:])
            hrelu = work.tile([128, R], bf16, tag="hrelu")
            nc.scalar.activation(hrelu[:], hsb[:],
                                 mybir.ActivationFunctionType.Relu,
                                 bias=b1c_sb[:])
            po = psum.tile([OUTW, R], f32, tag="head")
            nc.tensor.matmul(po[:], w2_sb[:], hrelu[:], start=True, stop=True)
            om = work.tile([OUTW, R], f32, tag="om")
            nc.vector.scalar_tensor_tensor(
                out=om[:], in0=po[:], scalar=b2_sb[:], in1=keep_sb[:],
                op0=mybir.AluOpType.add, op1=mybir.AluOpType.mult)
            nc.sync.dma_start(out[:], om[:])
    nc.compile()
    return nc


def _host_prep(preds, image, W_conv, b_conv, W1, b1, W2p, b2p, W2a, b2a, W2d, b2d):
    from ml_dtypes import bfloat16

    p = preds[0].astype(np.float32)
    score = p[:, 4] * p[:, 5]
    masked = np.where(score > CONF, score, -np.inf)
    idx = np.argsort(-masked, kind="stable")[:K]
    top_s = masked[idx]
    xy, wh = p[:, 0:2], p[:, 2:4]
    boxes = np.concatenate([xy - wh / 2, xy + wh / 2], axis=-1)
    b = boxes[idx]
    valid = top_s > CONF
    x1, y1, x2, y2 = b[:, 0], b[:, 1], b[:, 2], b[:, 3]
    area = (x2 - x1) * (y2 - y1)
    iw = np.clip(np.minimum(x2[:, None], x2[None, :]) - np.maximum(x1[:, None], x1[None, :]), 0, None)
    ih = np.clip(np.minimum(y2[:, None], y2[None, :]) - np.maximum(y1[:, None], y1[None, :]), 0, None)
    iou = iw * ih / (area[:, None] + area[None, :] - iw * ih + 1e-7)
    keep = valid.copy()
    for i in range(K):
        sup = np.any((iou[i, :i] > IOU) & keep[:i])
        keep[i] = keep[i] & ~sup

    xi = np.clip(np.round(b[:, 0]).astype(np.int32), 0, IMG - INP)
    yi = np.clip(np.round(b[:, 1]).astype(np.int32), 0, IMG - INP)
    img0 = image[0]
    pad = np.zeros((R, 3, 66, 66), np.float32)
    for r in range(K):
        pad[r, :, 1:65, 1:65] = img0[:, yi[r]:yi[r] + 64, xi[r]:xi[r] + 64]

    from numpy.lib.stride_tricks import sliding_window_view
    # patches[roi, c, y, x, ky, kx]
    patches = sliding_window_view(pad, (3, 3), axis=(2, 3))
    # cols_all[core][(ph,c,ky,kx), (m, sub, b, l, px, r)]
    # y = 8*core + 2*ph + sub ; x = 2*(16*b + px) + l
    # P: [core, ph, sub, c, ky, kx, m, r, b, px, l]
    Pv = patches.reshape(NM, 4, 3, 64, 2, 16, 2, 3, 3)  # m, r, c, y, b, px, l, ky, kx
    Pv = Pv.reshape(NM, 4, 3, 8, 4, 2, 2, 16, 2, 3, 3)  # m r c core ph sub b px l ky kx
    Pt = np.ascontiguousarray(
        Pv.transpose(3, 4, 2, 9, 10, 0, 5, 6, 8, 7, 1)
    )  # core, ph, c, ky, kx, m, sub, b, l, px, r
    cols_all = np.ones((8, 112, 38912), np.float32)
    cols_all[:, 0:108, :] = Pt.reshape(8, 108, 38912)
    cols_all = cols_all.astype(bfloat16)

    wstk = np.zeros((112, 64), np.float32)
    wc = W_conv.reshape(16, 27).T  # [27, 16]
    for ph in range(4):
        wstk[ph * 27:(ph + 1) * 27, ph * 16:(ph + 1) * 16] = wc
        wstk[108 + ph, ph * 16:(ph + 1) * 16] = b_conv
    wstk = wstk.astype(bfloat16)

    # w1s[core][h, p=(b,ph,oc), q=px_lo, d]
    W1r = W1.reshape(8, 16, 32, 2, 16, 128)  # h, oc, py, b, q, d
    w1s_all = np.empty((8, 8, 128, 16, 128), np.float32)
    for core in range(8):
        blk = W1r[:, :, 4 * core:4 * core + 4]            # h, oc, ph, b, q, d
        w1s_all[core] = blk.transpose(0, 3, 2, 1, 4, 5).reshape(8, 128, 16, 128)
    w1s_all = w1s_all.astype(bfloat16)

    w2_all = np.zeros((8, 128, OUTW), np.float32)
    b2_all = np.zeros((8, OUTW, 1), np.float32)
    w2_all[0, :, :PROV] = W2p; b2_all[0, :PROV, 0] = b2p
    w2_all[1, :, :ALPHA] = W2a; b2_all[1, :ALPHA, 0] = b2a
    for j in range(6):
        w2_all[2 + j, :, :AD] = W2d[j]; b2_all[2 + j, :AD, 0] = b2d[j]
    w2_all = w2_all.astype(bfloat16)

    keepf = np.zeros((R,), np.float32)
    keepf[:K] = keep.astype(np.float32)
    keepm = np.broadcast_to(keepf, (OUTW, R)).copy()

    in_maps = []
    for core in range(8):
        in_maps.append({
            "cols": cols_all[core],
            "wstk": wstk,
            "w1s": w1s_all[core],
            "b1c": b1[core].reshape(128, 1).astype(np.float32),
            "w2": w2_all[core],
            "b2": b2_all[core],
            "keepm": keepm,
        })
    return in_maps


def kernel(**inputs):
    from concourse import bass_utils
    if "nc" not in _CACHE:
        _CACHE["nc"] = _build_bass()
    nc = _CACHE["nc"]
    in_maps = _host_prep(**{k: np.asarray(v) for k, v in inputs.items()})
    res = bass_utils.run_bass_kernel_spmd(nc, in_maps, core_ids=list(range(N_CORES)))
    _CACHE["last_res"] = res
    outs = [res.results[c]["out"].T for c in range(N_CORES)]  # [304, 40] each
    logits = np.concatenate(
        [outs[0][:K, :PROV], outs[1][:K, :ALPHA]]
        + [outs[2 + j][:K, :AD] for j in range(6)], axis=1)
    return logits.astype(np.float32)


# revision 10
# speedup vs baseline: 1.8903x; 1.0540x over previous
"""Trainium2 Bass kernel for nn_CombinedModel (NMS detection + ROI classifier).

Sharding: pooled-pixel-row sharding. Core c computes conv output rows
y in [8c, 8c+8) (= pooled rows py in [4c,4c+4)) of ALL 300 ROIs, which is
exactly the k-slice S_c = {(oc, py, px): py in [4c,4c+4)} of the 16384-wide
W1 contraction. Each core runs the 8-head GEMM against its 2048-row W1
slice, a ReduceScatter sums the partial [8,128,304] and hands head c to
core c, which applies bias/relu + its head matmul + keep mask.
NMS / top-k / ROI selection is tiny and done host-side during input prep.

v2: bf16 matmuls (4x PE), conv bias folded into matmul via extra K rows,
pool-before-relu with relu fused into the x-pool scalar_tensor_tensor,
y-pool on GpSimd, contiguous pooling layout, bf16 ReduceScatter payload.
"""
import numpy as np

N_CORES = 8
R = 304            # 300 rois padded to 8*38
IMG = 640
INP = 64
CONF = 0.25
IOU = 0.45
K = 300
PROV, ALPHA, AD = 38, 25, 35
OUTW = 40          # padded per-core head width
NM = 76            # conv matmuls of 512 cols each (4 rois apiece)

_CACHE = {}


def _build_bass():
    import concourse.bacc as bacc
    import concourse.mybir as mybir
    import concourse.tile as tile

    nc = bacc.Bacc("TRN2", target_bir_lowering=False, debug=False,
                   num_devices=N_CORES)
    f32 = mybir.dt.float32
    bf16 = mybir.dt.bfloat16
    cols = nc.dram_tensor("cols", [112, 38912], bf16, kind="ExternalInput").ap()
    wstk = nc.dram_tensor("wstk", [112, 64], bf16, kind="ExternalInput").ap()
    w1s = nc.dram_tensor("w1s", [8, 128, 16, 128], bf16, kind="ExternalInput").ap()
    b1c = nc.dram_tensor("b1c", [128, 1], f32, kind="ExternalInput").ap()
    w2 = nc.dram_tensor("w2", [128, OUTW], bf16, kind="ExternalInput").ap()
    b2 = nc.dram_tensor("b2", [OUTW, 1], f32, kind="ExternalInput").ap()
    keepm = nc.dram_tensor("keepm", [OUTW, R], f32, kind="ExternalInput").ap()
    out = nc.dram_tensor("out", [OUTW, R], f32, kind="ExternalOutput").ap()

    with tile.TileContext(nc) as tc:
        with (
            tc.tile_pool(name="const", bufs=1) as cpool,
            tc.tile_pool(name="psum", bufs=1, space="PSUM") as psum,
            tc.tile_pool(name="work", bufs=2) as work,
            tc.tile_pool(name="dram", bufs=1, space="DRAM") as dpool,
        ):
            # small constants first (cheap, unblock tail setup)
            wstk_sb = cpool.tile([112, 64], bf16)
            nc.sync.dma_start(wstk_sb[:], wstk[:])
            b1c_sb = cpool.tile([128, 1], f32)
            nc.sync.dma_start(b1c_sb[:], b1c[:])
            w2_sb = cpool.tile([128, OUTW], bf16)
            nc.sync.dma_start(w2_sb[:], w2[:])
            b2_sb = cpool.tile([OUTW, 1], f32)
            nc.sync.dma_start(b2_sb[:], b2[:])
            keep_sb = cpool.tile([OUTW, R], f32)
            nc.sync.dma_start(keep_sb[:], keepm[:])

            # whole im2col matrix persistent in SBUF, DMA'd in col-chunks
            cols_sb = cpool.tile([112, 38912], bf16)
            NCH = 8
            CW = 38912 // NCH
            for ch in range(NCH):
                nc.sync.dma_start(cols_sb[:, ch * CW:(ch + 1) * CW],
                                  cols[:, ch * CW:(ch + 1) * CW])
            # W1 k-slice, one DMA per head so GEMM heads start as they land
            w1_sb = cpool.tile([128, 8, 16, 128], bf16)
            for h in range(8):
                nc.sync.dma_start(w1_sb[:, h], w1s[h])

            pooled2 = cpool.tile([128, 16, R], bf16)
            m01big = cpool.tile([64, NM * 256], bf16)
            m01bv = m01big.rearrange("p (m b l x r) -> p m b l x r",
                                     m=NM, b=2, l=2, x=16)

            # conv: matmul m covers 4 rois; PSUM free layout
            # (sub2, b2, l2, px16, r4); bias rides rows 108..111 of wstk/cols
            for m in range(NM):
                ps = psum.tile([64, 512], f32, tag="cv", bufs=4)
                nc.tensor.matmul(ps[:], wstk_sb[:],
                                 cols_sb[:, m * 512:(m + 1) * 512],
                                 start=True, stop=True)
                m0 = work.tile([64, 256], f32, tag="m0", bufs=3)
                nc.scalar.copy(m0[:], ps[:, 0:256])
                # y-pool + relu fused: max(ps1, 0, ps0) = relu(max(ps0, ps1))
                nc.vector.scalar_tensor_tensor(
                    out=m01big[:, m * 256:(m + 1) * 256],
                    in0=ps[:, 256:512], scalar=0.0, in1=m0[:],
                    op0=mybir.AluOpType.max, op1=mybir.AluOpType.max)
                # x-pool batched over 4-matmul groups
                if m % 4 == 3:
                    g = m // 4
                    for b in range(2):
                        nc.vector.tensor_tensor(
                            out=pooled2[64 * b:64 * b + 64, :,
                                        16 * g:16 * g + 16].rearrange(
                                "p x (m r) -> p m x r", m=4),
                            in0=m01bv[:, 4 * g:4 * g + 4, b, 0],
                            in1=m01bv[:, 4 * g:4 * g + 4, b, 1],
                            op=mybir.AluOpType.max)

            # 8-head GEMM over this core's 2048-row W1 slice
            cc_in_a = dpool.tile([8, 64, R], bf16)
            cc_in_b = dpool.tile([8, 64, R], bf16)
            cc_out_a = dpool.tile([64, R], bf16)
            cc_out_b = dpool.tile([64, R], bf16)
            for h in range(8):
                ph = psum.tile([128, R], f32, tag="gemm", bufs=3)
                for q in range(16):
                    nc.tensor.matmul(ph[:], w1_sb[:, h, q, :], pooled2[:, q, :],
                                     start=(q == 0), stop=(q == 15))
                pb = work.tile([128, R], bf16, tag="pb", bufs=2)
                nc.scalar.copy(pb[:], ph[:])
                nc.sync.dma_start(cc_in_a[h], pb[0:64, :])
                nc.sync.dma_start(cc_in_b[h], pb[64:128, :])

            # two RS halves -> may run on both CC queues concurrently
            nc.gpsimd.collective_compute(
                "ReduceScatter", mybir.AluOpType.add,
                ins=[cc_in_a[:]], outs=[cc_out_a[:]],
                replica_groups=[list(range(N_CORES))],
            )
            nc.gpsimd.collective_compute(
                "ReduceScatter", mybir.AluOpType.add,
                ins=[cc_in_b[:]], outs=[cc_out_b[:]],
                replica_groups=[list(range(N_CORES))],
            )
            hsb = work.tile([128, R], bf16, tag="hsb")
            nc.sync.dma_start(hsb[0:64, :], cc_out_a[:])
            nc.sync.dma_start(hsb[64:128, :], cc_out_b[:])
            hrelu = work.tile([128, R], bf16, tag="hrelu")
            nc.scalar.activation(hrelu[:], hsb[:],
                                 mybir.ActivationFunctionType.Relu,
                                 bias=b1c_sb[:])
            po = psum.tile([OUTW, R], f32, tag="head")
            nc.tensor.matmul(po[:], w2_sb[:], hrelu[:], start=True, stop=True)
            om = work.tile([OUTW, R], f32, tag="om")
            nc.vector.scalar_tensor_tensor(
                out=om[:], in0=po[:], scalar=b2_sb[:], in1=keep_sb[:],
                op0=mybir.AluOpType.add, op1=mybir.AluOpType.mult)
            nc.sync.dma_start(out[:], om[:])
    nc.compile()
    return nc


def _host_prep(preds, image, W_conv, b_conv, W1, b1, W2p, b2p, W2a, b2a, W2d, b2d):
    from ml_dtypes import bfloat16

    p = preds[0].astype(np.float32)
    score = p[:, 4] * p[:, 5]
    masked = np.where(score > CONF, score, -np.inf)
    idx = np.argsort(-masked, kind="stable")[:K]
    top_s = masked[idx]
    xy, wh = p[:, 0:2], p[:, 2:4]
    boxes = np.concatenate([xy - wh / 2, xy + wh / 2], axis=-1)
    b = boxes[idx]
    valid = top_s > CONF
    x1, y1, x2, y2 = b[:, 0], b[:, 1], b[:, 2], b[:, 3]
    area = (x2 - x1) * (y2 - y1)
    iw = np.clip(np.minimum(x2[:, None], x2[None, :]) - np.maximum(x1[:, None], x1[None, :]), 0, None)
    ih = np.clip(np.minimum(y2[:, None], y2[None, :]) - np.maximum(y1[:, None], y1[None, :]), 0, None)
    iou = iw * ih / (area[:, None] + area[None, :] - iw * ih + 1e-7)
    keep = valid.copy()
    for i in range(K):
        sup = np.any((iou[i, :i] > IOU) & keep[:i])
        keep[i] = keep[i] & ~sup

    xi = np.clip(np.round(b[:, 0]).astype(np.int32), 0, IMG - INP)
    yi = np.clip(np.round(b[:, 1]).astype(np.int32), 0, IMG - INP)
    img0 = image[0]
    pad = np.zeros((R, 3, 66, 66), np.float32)
    for r in range(K):
        pad[r, :, 1:65, 1:65] = img0[:, yi[r]:yi[r] + 64, xi[r]:xi[r] + 64]

    from numpy.lib.stride_tricks import sliding_window_view
    # patches[roi, c, y, x, ky, kx]
    patches = sliding_window_view(pad, (3, 3), axis=(2, 3))
    # cols_all[core][(ph,c,ky,kx), (m, sub, b, l, px, r)]
    # y = 8*core + 2*ph + sub ; x = 2*(16*b + px) + l
    # P: [core, ph, sub, c, ky, kx, m, r, b, px, l]
    Pv = patches.reshape(NM, 4, 3, 64, 2, 16, 2, 3, 3)  # m, r, c, y, b, px, l, ky, kx
    Pv = Pv.reshape(NM, 4, 3, 8, 4, 2, 2, 16, 2, 3, 3)  # m r c core ph sub b px l ky kx
    Pt = np.ascontiguousarray(
        Pv.transpose(3, 4, 2, 9, 10, 0, 5, 6, 8, 7, 1)
    )  # core, ph, c, ky, kx, m, sub, b, l, px, r
    cols_all = np.ones((8, 112, 38912), np.float32)
    cols_all[:, 0:108, :] = Pt.reshape(8, 108, 38912)
    cols_all = cols_all.astype(bfloat16)

    wstk = np.zeros((112, 64), np.float32)
    wc = W_conv.reshape(16, 27).T  # [27, 16]
    for ph in range(4):
        wstk[ph * 27:(ph + 1) * 27, ph * 16:(ph + 1) * 16] = wc
        wstk[108 + ph, ph * 16:(ph + 1) * 16] = b_conv
    wstk = wstk.astype(bfloat16)

    # w1s[core][h, p=(b,ph,oc), q=px_lo, d]
    W1r = W1.reshape(8, 16, 32, 2, 16, 128)  # h, oc, py, b, q, d
    w1s_all = np.empty((8, 8, 128, 16, 128), np.float32)
    for core in range(8):
        blk = W1r[:, :, 4 * core:4 * core + 4]            # h, oc, ph, b, q, d
        w1s_all[core] = blk.transpose(0, 3, 2, 1, 4, 5).reshape(8, 128, 16, 128)
    w1s_all = w1s_all.astype(bfloat16)

    w2_all = np.zeros((8, 128, OUTW), np.float32)
    b2_all = np.zeros((8, OUTW, 1), np.float32)
    w2_all[0, :, :PROV] = W2p; b2_all[0, :PROV, 0] = b2p
    w2_all[1, :, :ALPHA] = W2a; b2_all[1, :ALPHA, 0] = b2a
    for j in range(6):
        w2_all[2 + j, :, :AD] = W2d[j]; b2_all[2 + j, :AD, 0] = b2d[j]
    w2_all = w2_all.astype(bfloat16)

    keepf = np.zeros((R,), np.float32)
    keepf[:K] = keep.astype(np.float32)
    keepm = np.broadcast_to(keepf, (OUTW, R)).copy()

    in_maps = []
    for core in range(8):
        in_maps.append({
            "cols": cols_all[core],
            "wstk": wstk,
            "w1s": w1s_all[core],
            "b1c": b1[core].reshape(128, 1).astype(np.float32),
            "w2": w2_all[core],
            "b2": b2_all[core],
            "keepm": keepm,
        })
    return in_maps


def kernel(**inputs):
    from concourse import bass_utils
    if "nc" not in _CACHE:
        _CACHE["nc"] = _build_bass()
    nc = _CACHE["nc"]
    in_maps = _host_prep(**{k: np.asarray(v) for k, v in inputs.items()})
    res = bass_utils.run_bass_kernel_spmd(nc, in_maps, core_ids=list(range(N_CORES)))
    _CACHE["last_res"] = res
    outs = [res.results[c]["out"].T for c in range(N_CORES)]  # [304, 40] each
    logits = np.concatenate(
        [outs[0][:K, :PROV], outs[1][:K, :ALPHA]]
        + [outs[2 + j][:K, :AD] for j in range(6)], axis=1)
    return logits.astype(np.float32)


# revision 12
# speedup vs baseline: 2.2339x; 1.1818x over previous
"""Trainium2 Bass kernel for nn_CombinedModel (NMS detection + ROI classifier).

Sharding: pooled-pixel-row sharding. Core c computes conv output rows
y in [8c, 8c+8) (= pooled rows py in [4c,4c+4)) of ALL 300 ROIs, which is
exactly the k-slice S_c = {(oc, py, px): py in [4c,4c+4)} of the 16384-wide
W1 contraction. Each core runs the 8-head GEMM against its 2048-row W1
slice, a ReduceScatter sums the partial [8,128,304] and hands head c to
core c, which applies bias/relu + its head matmul + keep mask.
NMS / top-k / ROI selection is tiny and done host-side during input prep.

v2: bf16 matmuls (4x PE), conv bias folded into matmul via extra K rows,
pool-before-relu with relu fused into the x-pool scalar_tensor_tensor,
y-pool on GpSimd, contiguous pooling layout, bf16 ReduceScatter payload.
"""
import numpy as np

N_CORES = 8
R = 304            # 300 rois padded to 8*38
IMG = 640
INP = 64
CONF = 0.25
IOU = 0.45
K = 300
PROV, ALPHA, AD = 38, 25, 35
OUTW = 40          # padded per-core head width
NM = 76            # conv matmuls of 512 cols each (4 rois apiece)

_CACHE = {}


def _build_bass():
    import concourse.bacc as bacc
    import concourse.mybir as mybir
    import concourse.tile as tile

    nc = bacc.Bacc("TRN2", target_bir_lowering=False, debug=False,
                   num_devices=N_CORES)
    f32 = mybir.dt.float32
    bf16 = mybir.dt.bfloat16
    cols = nc.dram_tensor("cols", [112, 38912], bf16, kind="ExternalInput").ap()
    wstk = nc.dram_tensor("wstk", [112, 64], bf16, kind="ExternalInput").ap()
    w1s = nc.dram_tensor("w1s", [8, 128, 16, 128], bf16, kind="ExternalInput").ap()
    b1c = nc.dram_tensor("b1c", [128, 1], f32, kind="ExternalInput").ap()
    w2 = nc.dram_tensor("w2", [128, OUTW], bf16, kind="ExternalInput").ap()
    b2 = nc.dram_tensor("b2", [OUTW, 1], f32, kind="ExternalInput").ap()
    keepm = nc.dram_tensor("keepm", [OUTW, R], f32, kind="ExternalInput").ap()
    out = nc.dram_tensor("out", [OUTW, R], f32, kind="ExternalOutput").ap()

    with tile.TileContext(nc) as tc:
        with (
            tc.tile_pool(name="const", bufs=1) as cpool,
            tc.tile_pool(name="psum", bufs=1, space="PSUM") as psum,
            tc.tile_pool(name="work", bufs=2) as work,
            tc.tile_pool(name="dram", bufs=1, space="DRAM") as dpool,
        ):
            # small constants first (cheap, unblock tail setup)
            wstk_sb = cpool.tile([112, 64], bf16)
            nc.sync.dma_start(wstk_sb[:], wstk[:])
            b1c_sb = cpool.tile([128, 1], f32)
            nc.sync.dma_start(b1c_sb[:], b1c[:])
            w2_sb = cpool.tile([128, OUTW], bf16)
            nc.sync.dma_start(w2_sb[:], w2[:])
            b2_sb = cpool.tile([OUTW, 1], f32)
            nc.sync.dma_start(b2_sb[:], b2[:])
            keep_sb = cpool.tile([OUTW, R], f32)
            nc.sync.dma_start(keep_sb[:], keepm[:])

            # whole im2col matrix persistent in SBUF, DMA'd in col-chunks
            cols_sb = cpool.tile([112, 38912], bf16)
            NCH = 8
            CW = 38912 // NCH
            for ch in range(NCH):
                nc.sync.dma_start(cols_sb[:, ch * CW:(ch + 1) * CW],
                                  cols[:, ch * CW:(ch + 1) * CW])
            # W1 k-slice, one DMA per head so GEMM heads start as they land
            w1_sb = cpool.tile([128, 8, 16, 128], bf16)
            for h in range(8):
                nc.sync.dma_start(w1_sb[:, h], w1s[h])

            pooled2 = cpool.tile([128, 16, R], bf16)
            m01big = cpool.tile([64, NM * 256], bf16)
            m01bv = m01big.rearrange("p (m b l x r) -> p m b l x r",
                                     m=NM, b=2, l=2, x=16)

            # warmup collective: absorbs cross-core launch skew on the CC
            # queue while conv runs, so the real RS below waits ~nothing
            warm_in = dpool.tile([8, 2], f32)
            warm_out = dpool.tile([1, 2], f32)
            nc.sync.dma_start(warm_in[:], keepm[0:8, 0:2])
            nc.gpsimd.collective_compute(
                "ReduceScatter", mybir.AluOpType.add,
                ins=[warm_in[:]], outs=[warm_out[:]],
                replica_groups=[list(range(N_CORES))],
            )

            # conv: 2 matmuls per 2-bank PSUM tile; matmul m covers 4 rois;
            # PSUM free layout per mm: (sub2, b2, l2, px16, r4); bias rides
            # rows 108..111 of wstk/cols
            for mp in range(NM // 2):
                ps = psum.tile([64, 2, 2, 256], f32, tag="cv", bufs=2)
                for j in range(2):
                    m = 2 * mp + j
                    nc.tensor.matmul(ps[:, j].rearrange("p a b -> p (a b)"),
                                     wstk_sb[:],
                                     cols_sb[:, m * 512:(m + 1) * 512],
                                     start=True, stop=True)
                m0 = work.tile([64, 2, 256], f32, tag="m0", bufs=3)
                nc.scalar.copy(m0[:], ps[:, :, 0, :])
                # y-pool + relu fused: max(ps1, 0, ps0) = relu(max(ps0, ps1))
                nc.vector.scalar_tensor_tensor(
                    out=m01big[:, mp * 512:(mp + 1) * 512].rearrange(
                        "p (a b) -> p a b", a=2),
                    in0=ps[:, :, 1, :], scalar=0.0, in1=m0[:],
                    op0=mybir.AluOpType.max, op1=mybir.AluOpType.max)
                # x-pool batched over 4-matmul groups
                if mp % 2 == 1:
                    g = mp // 2
                    for b in range(2):
                        nc.vector.tensor_tensor(
                            out=pooled2[64 * b:64 * b + 64, :,
                                        16 * g:16 * g + 16].rearrange(
                                "p x (m r) -> p m x r", m=4),
                            in0=m01bv[:, 4 * g:4 * g + 4, b, 0],
                            in1=m01bv[:, 4 * g:4 * g + 4, b, 1],
                            op=mybir.AluOpType.max)

            # 8-head GEMM over this core's 2048-row W1 slice
            cc_in = dpool.tile([8, 128, R], bf16)
            cc_out = dpool.tile([128, R], bf16)
            for h in range(8):
                ph = psum.tile([128, R], f32, tag="gemm", bufs=3)
                for q in range(16):
                    nc.tensor.matmul(ph[:], w1_sb[:, h, q, :], pooled2[:, q, :],
                                     start=(q == 0), stop=(q == 15))
                pb = work.tile([128, R], bf16, tag="pb", bufs=2)
                nc.scalar.copy(pb[:], ph[:])
                nc.sync.dma_start(cc_in[h], pb[:])

            nc.gpsimd.collective_compute(
                "ReduceScatter", mybir.AluOpType.add,
                ins=[cc_in[:]], outs=[cc_out[:]],
                replica_groups=[list(range(N_CORES))],
            )
            hsb = work.tile([128, R], bf16, tag="hsb")
            nc.sync.dma_start(hsb[:], cc_out[:])
            hrelu = work.tile([128, R], bf16, tag="hrelu")
            nc.scalar.activation(hrelu[:], hsb[:],
                                 mybir.ActivationFunctionType.Relu,
                                 bias=b1c_sb[:])
            po = psum.tile([OUTW, R], f32, tag="head")
            nc.tensor.matmul(po[:], w2_sb[:], hrelu[:], start=True, stop=True)
            om = work.tile([OUTW, R], f32, tag="om")
            nc.vector.scalar_tensor_tensor(
                out=om[:], in0=po[:], scalar=b2_sb[:], in1=keep_sb[:],
                op0=mybir.AluOpType.add, op1=mybir.AluOpType.mult)
            nc.sync.dma_start(out[:], om[:])
    nc.compile()
    return nc


def _host_prep(preds, image, W_conv, b_conv, W1, b1, W2p, b2p, W2a, b2a, W2d, b2d):
    from ml_dtypes import bfloat16

    p = preds[0].astype(np.float32)
    score = p[:, 4] * p[:, 5]
    masked = np.where(score > CONF, score, -np.inf)
    idx = np.argsort(-masked, kind="stable")[:K]
    top_s = masked[idx]
    xy, wh = p[:, 0:2], p[:, 2:4]
    boxes = np.concatenate([xy - wh / 2, xy + wh / 2], axis=-1)
    b = boxes[idx]
    valid = top_s > CONF
    x1, y1, x2, y2 = b[:, 0], b[:, 1], b[:, 2], b[:, 3]
    area = (x2 - x1) * (y2 - y1)
    iw = np.clip(np.minimum(x2[:, None], x2[None, :]) - np.maximum(x1[:, None], x1[None, :]), 0, None)
    ih = np.clip(np.minimum(y2[:, None], y2[None, :]) - np.maximum(y1[:, None], y1[None, :]), 0, None)
    iou = iw * ih / (area[:, None] + area[None, :] - iw * ih + 1e-7)
    keep = valid.copy()
    for i in range(K):
        sup = np.any((iou[i, :i] > IOU) & keep[:i])
        keep[i] = keep[i] & ~sup

    xi = np.clip(np.round(b[:, 0]).astype(np.int32), 0, IMG - INP)
    yi = np.clip(np.round(b[:, 1]).astype(np.int32), 0, IMG - INP)
    img0 = image[0]
    pad = np.zeros((R, 3, 66, 66), np.float32)
    for r in range(K):
        pad[r, :, 1:65, 1:65] = img0[:, yi[r]:yi[r] + 64, xi[r]:xi[r] + 64]

    from numpy.lib.stride_tricks import sliding_window_view
    # patches[roi, c, y, x, ky, kx]
    patches = sliding_window_view(pad, (3, 3), axis=(2, 3))
    # cols_all[core][(ph,c,ky,kx), (m, sub, b, l, px, r)]
    # y = 8*core + 2*ph + sub ; x = 2*(16*b + px) + l
    # P: [core, ph, sub, c, ky, kx, m, r, b, px, l]
    Pv = patches.reshape(NM, 4, 3, 64, 2, 16, 2, 3, 3)  # m, r, c, y, b, px, l, ky, kx
    Pv = Pv.reshape(NM, 4, 3, 8, 4, 2, 2, 16, 2, 3, 3)  # m r c core ph sub b px l ky kx
    Pt = np.ascontiguousarray(
        Pv.transpose(3, 4, 2, 9, 10, 0, 5, 6, 8, 7, 1)
    )  # core, ph, c, ky, kx, m, sub, b, l, px, r
    cols_all = np.ones((8, 112, 38912), np.float32)
    cols_all[:, 0:108, :] = Pt.reshape(8, 108, 38912)
    cols_all = cols_all.astype(bfloat16)

    wstk = np.zeros((112, 64), np.float32)
    wc = W_conv.reshape(16, 27).T  # [27, 16]
    for ph in range(4):
        wstk[ph * 27:(ph + 1) * 27, ph * 16:(ph + 1) * 16] = wc
        wstk[108 + ph, ph * 16:(ph + 1) * 16] = b_conv
    wstk = wstk.astype(bfloat16)

    # w1s[core][h, p=(b,ph,oc), q=px_lo, d]
    W1r = W1.reshape(8, 16, 32, 2, 16, 128)  # h, oc, py, b, q, d
    w1s_all = np.empty((8, 8, 128, 16, 128), np.float32)
    for core in range(8):
        blk = W1r[:, :, 4 * core:4 * core + 4]            # h, oc, ph, b, q, d
        w1s_all[core] = blk.transpose(0, 3, 2, 1, 4, 5).reshape(8, 128, 16, 128)
    w1s_all = w1s_all.astype(bfloat16)

    w2_all = np.zeros((8, 128, OUTW), np.float32)
    b2_all = np.zeros((8, OUTW, 1), np.float32)
    w2_all[0, :, :PROV] = W2p; b2_all[0, :PROV, 0] = b2p
    w2_all[1, :, :ALPHA] = W2a; b2_all[1, :ALPHA, 0] = b2a
    for j in range(6):
        w2_all[2 + j, :, :AD] = W2d[j]; b2_all[2 + j, :AD, 0] = b2d[j]
    w2_all = w2_all.astype(bfloat16)

    keepf = np.zeros((R,), np.float32)
    keepf[:K] = keep.astype(np.float32)
    keepm = np.broadcast_to(keepf, (OUTW, R)).copy()

    in_maps = []
    for core in range(8):
        in_maps.append({
            "cols": cols_all[core],
            "wstk": wstk,
            "w1s": w1s_all[core],
            "b1c": b1[core].reshape(128, 1).astype(np.float32),
            "w2": w2_all[core],
            "b2": b2_all[core],
            "keepm": keepm,
        })
    return in_maps


def kernel(**inputs):
    from concourse import bass_utils
    if "nc" not in _CACHE:
        _CACHE["nc"] = _build_bass()
    nc = _CACHE["nc"]
    in_maps = _host_prep(**{k: np.asarray(v) for k, v in inputs.items()})
    res = bass_utils.run_bass_kernel_spmd(nc, in_maps, core_ids=list(range(N_CORES)))
    _CACHE["last_res"] = res
    outs = [res.results[c]["out"].T for c in range(N_CORES)]  # [304, 40] each
    logits = np.concatenate(
        [outs[0][:K, :PROV], outs[1][:K, :ALPHA]]
        + [outs[2 + j][:K, :AD] for j in range(6)], axis=1)
    return logits.astype(np.float32)


# revision 19
# speedup vs baseline: 2.4925x; 1.1158x over previous
"""Trainium2 Bass kernel for nn_CombinedModel (NMS detection + ROI classifier).

Sharding: pooled-pixel-row sharding. Core c computes conv output rows
y in [8c, 8c+8) (= pooled rows py in [4c,4c+4)) of ALL 300 ROIs, which is
exactly the k-slice S_c = {(oc, py, px): py in [4c,4c+4)} of the 16384-wide
W1 contraction. Each core runs the 8-head GEMM against its 2048-row W1
slice, a ReduceScatter sums the partial [8,128,304] and hands head c to
core c, which applies bias/relu + its head matmul + keep mask.
NMS / top-k / ROI selection is tiny and done host-side during input prep.

v2: bf16 matmuls (4x PE), conv bias folded into matmul via extra K rows,
pool-before-relu with relu fused into the x-pool scalar_tensor_tensor,
y-pool on GpSimd, contiguous pooling layout, bf16 ReduceScatter payload.
"""
import numpy as np

N_CORES = 8
R = 304            # 300 rois padded to 8*38
IMG = 640
INP = 64
CONF = 0.25
IOU = 0.45
K = 300
PROV, ALPHA, AD = 38, 25, 35
OUTW = 40          # padded per-core head width
NM = 76            # conv matmuls of 512 cols each (4 rois apiece)

_CACHE = {}


def _build_bass():
    import concourse.bacc as bacc
    import concourse.mybir as mybir
    import concourse.tile as tile

    nc = bacc.Bacc("TRN2", target_bir_lowering=False, debug=False,
                   num_devices=N_CORES)
    f32 = mybir.dt.float32
    bf16 = mybir.dt.bfloat16
    cols = nc.dram_tensor("cols", [112, 38912], bf16, kind="ExternalInput").ap()
    wstk = nc.dram_tensor("wstk", [112, 64], bf16, kind="ExternalInput").ap()
    w1s = nc.dram_tensor("w1s", [8, 128, 16, 128], bf16, kind="ExternalInput").ap()
    b1c = nc.dram_tensor("b1c", [128, 1], f32, kind="ExternalInput").ap()
    w2 = nc.dram_tensor("w2", [128, OUTW], bf16, kind="ExternalInput").ap()
    b2 = nc.dram_tensor("b2", [OUTW, 1], f32, kind="ExternalInput").ap()
    keepm = nc.dram_tensor("keepm", [OUTW, R], f32, kind="ExternalInput").ap()
    out = nc.dram_tensor("out", [OUTW, R], f32, kind="ExternalOutput").ap()

    with tile.TileContext(nc) as tc:
        with (
            tc.tile_pool(name="const", bufs=1) as cpool,
            tc.tile_pool(name="psum", bufs=1, space="PSUM") as psum,
            tc.tile_pool(name="work", bufs=2) as work,
            tc.tile_pool(name="dram", bufs=1, space="DRAM") as dpool,
        ):
            # warmup collective first: absorbs cross-core launch skew on the
            # CC queue while conv runs, so the real RS below waits ~nothing
            warm_in = dpool.tile([8, 2], f32)
            warm_out = dpool.tile([1, 2], f32)
            nc.sync.dma_start(warm_in[:], keepm[0:8, 0:2])
            nc.gpsimd.collective_compute(
                "ReduceScatter", mybir.AluOpType.add,
                ins=[warm_in[:]], outs=[warm_out[:]],
                replica_groups=[list(range(N_CORES))],
            )

            # small constants first (cheap, unblock tail setup)
            wstk_sb = cpool.tile([112, 64], bf16)
            nc.sync.dma_start(wstk_sb[:], wstk[:])
            b1c_sb = cpool.tile([128, 1], f32)
            nc.sync.dma_start(b1c_sb[:], b1c[:])
            w2_sb = cpool.tile([128, OUTW], bf16)
            nc.sync.dma_start(w2_sb[:], w2[:])
            b2_sb = cpool.tile([OUTW, 1], f32)
            nc.sync.dma_start(b2_sb[:], b2[:])
            keep_sb = cpool.tile([OUTW, R], f32)
            nc.sync.dma_start(keep_sb[:], keepm[:])

            # whole im2col matrix persistent in SBUF, DMA'd in col-chunks
            cols_sb = cpool.tile([112, 38912], bf16)
            NCH = 8
            CW = 38912 // NCH
            for ch in range(NCH):
                nc.sync.dma_start(cols_sb[:, ch * CW:(ch + 1) * CW],
                                  cols[:, ch * CW:(ch + 1) * CW])
            # W1 k-slice, one DMA per head so GEMM heads start as they land
            w1_sb = cpool.tile([128, 8, 16, 128], bf16)
            for h in range(8):
                nc.sync.dma_start(w1_sb[:, h], w1s[h])

            pooled2 = cpool.tile([128, 16, R], bf16)
            m01big = cpool.tile([64, NM * 256], bf16)
            m01bv = m01big.rearrange("p (m b l x r) -> p m b l x r",
                                     m=NM, b=2, l=2, x=16)

            # conv: 2 matmuls per 2-bank PSUM tile; matmul m covers 4 rois;
            # PSUM free layout per mm: (sub2, b2, l2, px16, r4); bias rides
            # rows 108..111 of wstk/cols
            for mp in range(NM // 2):
                ps = psum.tile([64, 2, 2, 256], f32, tag="cv", bufs=3)
                for j in range(2):
                    m = 2 * mp + j
                    nc.tensor.matmul(ps[:, j].rearrange("p a b -> p (a b)"),
                                     wstk_sb[:],
                                     cols_sb[:, m * 512:(m + 1) * 512],
                                     start=True, stop=True)
                m0 = work.tile([64, 2, 256], f32, tag="m0", bufs=4)
                nc.scalar.copy(m0[:], ps[:, :, 0, :])
                # y-pool + relu fused: max(ps1, 0, ps0) = relu(max(ps0, ps1))
                nc.vector.scalar_tensor_tensor(
                    out=m01big[:, mp * 512:(mp + 1) * 512].rearrange(
                        "p (a b) -> p a b", a=2),
                    in0=ps[:, :, 1, :], scalar=0.0, in1=m0[:],
                    op0=mybir.AluOpType.max, op1=mybir.AluOpType.max)
                # x-pool batched over 4-matmul groups
                if mp % 2 == 1:
                    g = mp // 2
                    for b in range(2):
                        nc.vector.tensor_tensor(
                            out=pooled2[64 * b:64 * b + 64, :,
                                        16 * g:16 * g + 16].rearrange(
                                "p x (m r) -> p m x r", m=4),
                            in0=m01bv[:, 4 * g:4 * g + 4, b, 0],
                            in1=m01bv[:, 4 * g:4 * g + 4, b, 1],
                            op=mybir.AluOpType.max)

            # 8-head GEMM over this core's 2048-row W1 slice
            cc_in = dpool.tile([8, 128, R], bf16)
            cc_out = dpool.tile([128, R], bf16)
            for h in range(8):
                ph = psum.tile([128, R], f32, tag="gemm", bufs=2)
                for q in range(16):
                    nc.tensor.matmul(ph[:], w1_sb[:, h, q, :], pooled2[:, q, :],
                                     start=(q == 0), stop=(q == 15))
                pb = work.tile([128, R], bf16, tag="pb", bufs=2)
                nc.vector.tensor_copy(pb[:], ph[:])
                nc.sync.dma_start(cc_in[h], pb[:])

            nc.gpsimd.collective_compute(
                "ReduceScatter", mybir.AluOpType.add,
                ins=[cc_in[:]], outs=[cc_out[:]],
                replica_groups=[list(range(N_CORES))],
            )
            hsb = work.tile([128, R], bf16, tag="hsb")
            nc.sync.dma_start(hsb[:], cc_out[:])
            hrelu = work.tile([128, R], bf16, tag="hrelu")
            nc.scalar.activation(hrelu[:], hsb[:],
                                 mybir.ActivationFunctionType.Relu,
                                 bias=b1c_sb[:])
            po = psum.tile([128, R], f32, tag="gemm", bufs=2)
            nc.tensor.matmul(po[0:OUTW, :], w2_sb[:], hrelu[:],
                             start=True, stop=True)
            om = work.tile([OUTW, R], f32, tag="om")
            nc.vector.scalar_tensor_tensor(
                out=om[:], in0=po[0:OUTW, :], scalar=b2_sb[:], in1=keep_sb[:],
                op0=mybir.AluOpType.add, op1=mybir.AluOpType.mult)
            nc.sync.dma_start(out[:], om[:])
    nc.compile()
    return nc


def _host_prep(preds, image, W_conv, b_conv, W1, b1, W2p, b2p, W2a, b2a, W2d, b2d):
    from ml_dtypes import bfloat16

    p = preds[0].astype(np.float32)
    score = p[:, 4] * p[:, 5]
    masked = np.where(score > CONF, score, -np.inf)
    idx = np.argsort(-masked, kind="stable")[:K]
    top_s = masked[idx]
    xy, wh = p[:, 0:2], p[:, 2:4]
    boxes = np.concatenate([xy - wh / 2, xy + wh / 2], axis=-1)
    b = boxes[idx]
    valid = top_s > CONF
    x1, y1, x2, y2 = b[:, 0], b[:, 1], b[:, 2], b[:, 3]
    area = (x2 - x1) * (y2 - y1)
    iw = np.clip(np.minimum(x2[:, None], x2[None, :]) - np.maximum(x1[:, None], x1[None, :]), 0, None)
    ih = np.clip(np.minimum(y2[:, None], y2[None, :]) - np.maximum(y1[:, None], y1[None, :]), 0, None)
    iou = iw * ih / (area[:, None] + area[None, :] - iw * ih + 1e-7)
    keep = valid.copy()
    for i in range(K):
        sup = np.any((iou[i, :i] > IOU) & keep[:i])
        keep[i] = keep[i] & ~sup

    xi = np.clip(np.round(b[:, 0]).astype(np.int32), 0, IMG - INP)
    yi = np.clip(np.round(b[:, 1]).astype(np.int32), 0, IMG - INP)
    img0 = image[0]
    pad = np.zeros((R, 3, 66, 66), np.float32)
    for r in range(K):
        pad[r, :, 1:65, 1:65] = img0[:, yi[r]:yi[r] + 64, xi[r]:xi[r] + 64]

    from numpy.lib.stride_tricks import sliding_window_view
    # patches[roi, c, y, x, ky, kx]
    patches = sliding_window_view(pad, (3, 3), axis=(2, 3))
    # cols_all[core][(ph,c,ky,kx), (m, sub, b, l, px, r)]
    # y = 8*core + 2*ph + sub ; x = 2*(16*b + px) + l
    # P: [core, ph, sub, c, ky, kx, m, r, b, px, l]
    Pv = patches.reshape(NM, 4, 3, 64, 2, 16, 2, 3, 3)  # m, r, c, y, b, px, l, ky, kx
    Pv = Pv.reshape(NM, 4, 3, 8, 4, 2, 2, 16, 2, 3, 3)  # m r c core ph sub b px l ky kx
    Pt = np.ascontiguousarray(
        Pv.transpose(3, 4, 2, 9, 10, 0, 5, 6, 8, 7, 1)
    )  # core, ph, c, ky, kx, m, sub, b, l, px, r
    cols_all = np.ones((8, 112, 38912), np.float32)
    cols_all[:, 0:108, :] = Pt.reshape(8, 108, 38912)
    cols_all = cols_all.astype(bfloat16)

    wstk = np.zeros((112, 64), np.float32)
    wc = W_conv.reshape(16, 27).T  # [27, 16]
    for ph in range(4):
        wstk[ph * 27:(ph + 1) * 27, ph * 16:(ph + 1) * 16] = wc
        wstk[108 + ph, ph * 16:(ph + 1) * 16] = b_conv
    wstk = wstk.astype(bfloat16)

    # w1s[core][h, p=(b,ph,oc), q=px_lo, d]
    W1r = W1.reshape(8, 16, 32, 2, 16, 128)  # h, oc, py, b, q, d
    w1s_all = np.empty((8, 8, 128, 16, 128), np.float32)
    for core in range(8):
        blk = W1r[:, :, 4 * core:4 * core + 4]            # h, oc, ph, b, q, d
        w1s_all[core] = blk.transpose(0, 3, 2, 1, 4, 5).reshape(8, 128, 16, 128)
    w1s_all = w1s_all.astype(bfloat16)

    w2_all = np.zeros((8, 128, OUTW), np.float32)
    b2_all = np.zeros((8, OUTW, 1), np.float32)
    w2_all[0, :, :PROV] = W2p; b2_all[0, :PROV, 0] = b2p
    w2_all[1, :, :ALPHA] = W2a; b2_all[1, :ALPHA, 0] = b2a
    for j in range(6):
        w2_all[2 + j, :, :AD] = W2d[j]; b2_all[2 + j, :AD, 0] = b2d[j]
    w2_all = w2_all.astype(bfloat16)

    keepf = np.zeros((R,), np.float32)
    keepf[:K] = keep.astype(np.float32)
    keepm = np.broadcast_to(keepf, (OUTW, R)).copy()

    in_maps = []
    for core in range(8):
        in_maps.append({
            "cols": cols_all[core],
            "wstk": wstk,
            "w1s": w1s_all[core],
            "b1c": b1[core].reshape(128, 1).astype(np.float32),
            "w2": w2_all[core],
            "b2": b2_all[core],
            "keepm": keepm,
        })
    return in_maps


def kernel(**inputs):
    from concourse import bass_utils
    if "nc" not in _CACHE:
        _CACHE["nc"] = _build_bass()
    nc = _CACHE["nc"]
    in_maps = _host_prep(**{k: np.asarray(v) for k, v in inputs.items()})
    res = bass_utils.run_bass_kernel_spmd(nc, in_maps, core_ids=list(range(N_CORES)))
    _CACHE["last_res"] = res
    outs = [res.results[c]["out"].T for c in range(N_CORES)]  # [304, 40] each
    logits = np.concatenate(
        [outs[0][:K, :PROV], outs[1][:K, :ALPHA]]
        + [outs[2 + j][:K, :AD] for j in range(6)], axis=1)
    return logits.astype(np.float32)
